# revision 13
# baseline (speedup 1.0000x reference)
"""Fused attention + FC + residual + LayerNorm for Trainium2, 8 NeuronCores.

Problem: B=8, L=2048, d_k=d_v=64, d_model=1024, fp32 I/O.
Sharding: pure data parallel - batch element b -> core b. No collectives.

This target's PE streams at 1.2 GHz (HAM never unthrottles), so matmul cost
is ~0.82 ns/column + ~73 ns/instruction; the kernel therefore row-packs the
K=64 matmuls (two concurrent matmuls in row groups 0-63 / 64-127 via
tile_position) at the cost of duplicating qT/kT (and fc_wT when fc_b == 0)
across both partition halves.

Software pipeline, one q-slice deep (engine queues are strict FIFO):

  iter s:  attention(s)  ->  epilogue(s-1)  ->  denominator-dance(s)

  attention(s): per k-tile pair, S^T [128k, 2x512q] via one row-packed
    matmul pair (bf16, f32 PSUM) -> exp on ScalarE (temperature 1/8 folded
    into the free affine scale, bf16 out) -> PV matmul accumulates [65, 512]
    f32, row 64 = softmax denominator (ones-column appended to V).
  dance(s): denominator row -> per-partition [128, 4] via 4 tiny K=1 PE
    matmuls -> reciprocal on DVE -> DMA to DRAM + broadcast-DMA back as
    [65, 512] (on the ScalarE HWDGE ring so it never queues behind bulk
    traffic) -> PSUM evacuation fused with the normalize as tensor_tensor
    multiplies (bf16 out). Row 64 becomes exactly denom*recip = 1, the bias
    row the K=65-augmented FC needs (used when fc_b != 0).
  epilogue(s): FC matmul -> DVE adds residual from PSUM -> bn_stats/bn_aggr
    -> rsqrt batched per slice as Ln/Exp on ScalarE (one ACT table set for
    the whole kernel) -> LN apply alternating DVE tensor_scalar / ScalarE
    Identity -> store on the GPSIMD SWDGE ring (keeps the sync ring free
    for loads).

  qT/kT [64->128 dup, 2048] and fc_wT are built by PE pair-transposes from
  tile-major loads; the PSUM evacuations double as the f32->bf16 cast.
"""
import numpy as np

B = 8
L = 2048
D = 64
DM = 1024
NTILES = L // 128       # 16 q/k tiles of 128
NSLICES = L // 512      # 4 q-slices of 512
LN_EPS = 1e-5
SCALE = 0.125           # 1/sqrt(64)

_CACHE = {}
_TABLES_PATCHED = False


def _patch_act_tables():
    """Force every activation we use into one table set so the scheduler
    never needs a mid-kernel ACT_TABLE_LOAD switch (Exp <-> Ln)."""
    global _TABLES_PATCHED
    if _TABLES_PATCHED:
        return
    import concourse.bacc as bacc
    from concourse import mybir

    orig = bacc.get_activation_tables
    keep = "natural_log_exp_and_others"
    shared = {
        mybir.ActivationFunctionType.Exp,
        mybir.ActivationFunctionType.Ln,
        mybir.ActivationFunctionType.Copy,
        mybir.ActivationFunctionType.Identity,
        mybir.ActivationFunctionType.Square,
    }

    def patched(arch):
        tables = orig(arch)
        for name, fns in tables.items():
            if name != keep:
                fns.difference_update(shared)
        return tables

    bacc.get_activation_tables = patched
    _TABLES_PATCHED = True


def _build(affine: bool, packed_fc: bool):
    import concourse.bacc as bacc
    import concourse.tile as tile
    from concourse import mybir
    import concourse.bass as bass
    from concourse.masks import make_identity

    _patch_act_tables()
    f32 = mybir.dt.float32
    bf16 = mybir.dt.bfloat16
    nc = bacc.Bacc("TRN2", target_bir_lowering=False, debug=False, num_devices=B)

    q_d = nc.declare_dram_parameter("q", [L, D], f32, isOutput=False)
    k_d = nc.declare_dram_parameter("k", [L, D], f32, isOutput=False)
    v_d = nc.declare_dram_parameter("v", [L, D], f32, isOutput=False)
    res_d = nc.declare_dram_parameter("residual", [L, DM], f32, isOutput=False)
    fcw_d = nc.declare_dram_parameter("fc_w", [DM, D], f32, isOutput=False)
    fcb_d = nc.declare_dram_parameter("fc_b", [DM], f32, isOutput=False)
    gam_d = nc.declare_dram_parameter("ln_gamma", [DM], f32, isOutput=False)
    bet_d = nc.declare_dram_parameter("ln_beta", [DM], f32, isOutput=False)
    out_d = nc.declare_dram_parameter("out", [L, DM], f32, isOutput=True)

    recip_s = nc.dram_tensor("recip_scratch", [L], f32)

    with tile.TileContext(nc) as tc:
        with (
            tc.tile_pool(name="raw", bufs=2) as raw_pool,
            tc.tile_pool(name="persist", bufs=1) as persist,
            tc.tile_pool(name="stage", bufs=2, space="PSUM") as stage_pool,
            tc.tile_pool(name="pv", bufs=2, space="PSUM") as pv_pool,
            tc.tile_pool(name="fc", bufs=1, space="PSUM") as fc_pool,
            tc.tile_pool(name="et", bufs=6) as et_pool,
            tc.tile_pool(name="resid", bufs=6) as res_pool,
            tc.tile_pool(name="x", bufs=8) as x_pool,
            tc.tile_pool(name="outs", bufs=4) as out_pool,
            tc.tile_pool(name="norm", bufs=2) as norm_pool,
            tc.tile_pool(name="small", bufs=4) as small_pool,
        ):
            identity = persist.tile([128, 128], f32)
            make_identity(nc, identity)
            eps_t = persist.tile([128, 1], f32, tag="eps")
            nc.vector.memset(eps_t, LN_EPS)
            one_c = persist.tile([1, 1], f32, tag="onec")
            nc.vector.memset(one_c, 1.0)

            # ---- tile-major loads + PE pair-transposes ----
            # qT2/kT2 [128, 16, 128] bf16: rows 0:63 = transposed data,
            # rows 64:127 = duplicate (for row-packed K=64 matmuls).
            # tile index = grp*8 + pair*2 + par
            qT2 = persist.tile([128, NTILES, 128], bf16, tag="qT")
            kT2 = persist.tile([128, NTILES, 128], bf16, tag="kT")
            for src, dstT in ((q_d, qT2), (k_d, kT2)):
                raw = raw_pool.tile([128, NTILES, D], f32, tag="raw")
                nc.sync.dma_start(
                    out=raw, in_=src.ap().rearrange("(t p) d -> p t d", p=128)
                )
                dlo = dstT[0:64, :, :].rearrange(
                    "d (grp pair par) c -> d grp pair par c", pair=4, par=2)
                dhi = dstT[64:128, :, :].rearrange(
                    "d (grp pair par) c -> d grp pair par c", pair=4, par=2)
                for grp in range(NTILES // 8):
                    pt = stage_pool.tile([128, 512], f32, tag="stage")
                    for i in range(4):
                        nc.tensor.transpose(
                            pt[:, i * 128:(i + 1) * 128],
                            raw[:, (8 * grp + 2 * i): (8 * grp + 2 * i + 2), :],
                            identity,
                        )
                    ptv = pt.rearrange("p (four c) -> p four c", c=128)
                    nc.vector.tensor_copy(dlo[:, grp, :, 0, :], ptv[0:64])
                    nc.vector.tensor_copy(dlo[:, grp, :, 1, :], ptv[64:128])
                    nc.vector.tensor_copy(dhi[:, grp, :, 0, :], ptv[0:64])
                    nc.vector.tensor_copy(dhi[:, grp, :, 1, :], ptv[64:128])

            # ---- v with ones column: [128, 16, 65] bf16 (tile-major) ----
            vraw = raw_pool.tile([128, NTILES, D], f32, tag="raw")
            nc.sync.dma_start(
                out=vraw, in_=v_d.ap().rearrange("(t p) d -> p t d", p=128)
            )
            v_sb = persist.tile([128, NTILES, D + 1], bf16, tag="v")
            nc.scalar.copy(v_sb[:, :, 0:D], vraw)
            nc.vector.memset(v_sb[:, :, D:D + 1], 1.0)

            # ---- fc_wT [65 or 128-dup, 1024] bf16 ----
            # packed_fc: rows 0:63 / 64:127 both hold fc_wT (row-packed FC,
            # fc_b known zero). else: rows 0:64 with row 64 = fc_b.
            fcwT = persist.tile([128 if packed_fc else 65, DM], bf16, tag="fcw")
            fraw = raw_pool.tile([128, DM // 128, D], f32, tag="raw")
            nc.sync.dma_start(
                out=fraw, in_=fcw_d.ap().rearrange("(t p) d -> p t d", p=128)
            )
            flo = fcwT[0:64, :].rearrange("d (pair par c) -> d pair par c",
                                          par=2, c=128)
            pt = stage_pool.tile([128, 512], f32, tag="stage")
            for i in range(4):
                nc.tensor.transpose(
                    pt[:, i * 128:(i + 1) * 128],
                    fraw[:, 2 * i: 2 * i + 2, :],
                    identity,
                )
            ptv = pt.rearrange("p (four c) -> p four c", c=128)
            nc.vector.tensor_copy(flo[:, :, 0, :], ptv[0:64])
            nc.vector.tensor_copy(flo[:, :, 1, :], ptv[64:128])
            if packed_fc:
                fhi = fcwT[64:128, :].rearrange(
                    "d (pair par c) -> d pair par c", par=2, c=128)
                nc.vector.tensor_copy(fhi[:, :, 0, :], ptv[0:64])
                nc.vector.tensor_copy(fhi[:, :, 1, :], ptv[64:128])
            else:
                fcb_t = small_pool.tile([1, DM], f32, tag="fcb")
                nc.sync.dma_start(
                    out=fcb_t,
                    in_=bass.AP(tensor=fcb_d, offset=0, ap=[[0, 1], [1, DM]]),
                )
                nc.vector.tensor_copy(fcwT[64:65, :], fcb_t)

            if affine:
                gam_bc = persist.tile([128, DM], f32, tag="gam")
                bet_bc = persist.tile([128, DM], f32, tag="bet")
                nc.sync.dma_start(
                    out=gam_bc,
                    in_=bass.AP(tensor=gam_d, offset=0, ap=[[0, 128], [1, DM]]),
                )
                nc.sync.dma_start(
                    out=bet_bc,
                    in_=bass.AP(tensor=bet_d, offset=0, ap=[[0, 128], [1, DM]]),
                )

            state = {}

            def attention(s):
                qlo = qT2[0:64, :, :].rearrange("d t c -> d (t c)")[
                    :, s * 512:(s + 1) * 512]
                qhi = qT2[64:128, :, :].rearrange("d t c -> d (t c)")[
                    :, s * 512:(s + 1) * 512]
                out_aug = pv_pool.tile([65, 512], f32, tag="pv")
                for g in range(NTILES // 2):
                    st = stage_pool.tile([128, 1024], f32, tag="stage")
                    # row-packed pair: k-tile 2g on rows 0:63,
                    # k-tile 2g+1 on rows 64:127 - concurrent on the PE
                    nc.tensor.matmul(st[:, 0:512], kT2[0:64, 2 * g, :], qlo,
                                     start=True, stop=True,
                                     tile_position=(0, 0))
                    nc.tensor.matmul(st[:, 512:1024], kT2[64:128, 2 * g + 1, :],
                                     qhi, start=True, stop=True,
                                     tile_position=(64, 0))
                    et = et_pool.tile([128, 1024], bf16, tag="et")
                    nc.scalar.activation(
                        out=et, in_=st,
                        func=mybir.ActivationFunctionType.Exp, scale=SCALE,
                    )
                    nc.tensor.matmul(out_aug, v_sb[:, 2 * g, :], et[:, 0:512],
                                     start=(g == 0), stop=False)
                    nc.tensor.matmul(out_aug, v_sb[:, 2 * g + 1, :],
                                     et[:, 512:1024],
                                     start=False, stop=(g == NTILES // 2 - 1))
                return out_aug

            def dance(s, out_aug):
                # denom row -> per-partition [128, 4] via 4 tiny PE matmuls
                drow = small_pool.tile([1, 512], f32, tag="drow")
                nc.scalar.copy(drow, out_aug[64:65, :])
                dT = stage_pool.tile([128, 4], f32, tag="stage")
                for t in range(4):
                    nc.tensor.matmul(dT[:, t:t + 1],
                                     drow[:, t * 128:(t + 1) * 128], one_c,
                                     start=True, stop=True)
                rT = small_pool.tile([128, 4], f32, tag="rT")
                nc.vector.reciprocal(rT, dT)
                nc.scalar.dma_start(
                    out=bass.AP(tensor=recip_s, offset=s * 512,
                                ap=[[1, 128], [128, 4]]),
                    in_=rT,
                )
                nparts = 128 if packed_fc else 65
                rbc = norm_pool.tile([nparts, 512], f32, tag="rbc")
                nc.scalar.dma_start(
                    out=rbc,
                    in_=bass.AP(tensor=recip_s, offset=s * 512,
                                ap=[[0, nparts], [1, 512]]),
                )
                # normalize + evacuate PSUM + cast bf16
                outT = norm_pool.tile([nparts, 512], bf16, tag="outT")
                if packed_fc:
                    # duplicate into both halves for row-packed FC
                    nc.vector.tensor_mul(outT[0:64, :], out_aug[0:64, :],
                                         rbc[0:64, :])
                    nc.vector.tensor_mul(outT[64:128, :], out_aug[0:64, :],
                                         rbc[64:128, :])
                else:
                    # row 64 -> denom*recip = 1.0 exactly (FC bias row)
                    nc.vector.tensor_mul(outT, out_aug, rbc)
                state[s] = {"outT": outT}

            def epilogue(s):
                outT = state[s]["outT"]
                mv_all = small_pool.tile([128, 4, 2], f32, tag="mv")
                x_ts = []
                for pi in range(4):
                    t = s * 4 + pi
                    fc_ps = fc_pool.tile([128, DM], f32, tag="fc")
                    if packed_fc:
                        nc.tensor.matmul(fc_ps[:, 0:512],
                                         outT[0:64, pi * 128:(pi + 1) * 128],
                                         fcwT[0:64, 0:512],
                                         start=True, stop=True,
                                         tile_position=(0, 0))
                        nc.tensor.matmul(fc_ps[:, 512:1024],
                                         outT[64:128, pi * 128:(pi + 1) * 128],
                                         fcwT[64:128, 512:1024],
                                         start=True, stop=True,
                                         tile_position=(64, 0))
                    else:
                        lhs = outT[:, pi * 128:(pi + 1) * 128]
                        nc.tensor.matmul(fc_ps[:, 0:512], lhs, fcwT[:, 0:512],
                                         start=True, stop=True)
                        nc.tensor.matmul(fc_ps[:, 512:1024], lhs,
                                         fcwT[:, 512:1024],
                                         start=True, stop=True)
                    res_t = res_pool.tile([128, DM], f32, tag="res")
                    nc.sync.dma_start(
                        out=res_t, in_=res_d[t * 128:(t + 1) * 128, :]
                    )
                    x_t = x_pool.tile([128, DM], f32, tag="x")
                    nc.vector.tensor_add(x_t, fc_ps, res_t)
                    x_ts.append(x_t)
                    stats = small_pool.tile([128, 2, 6], f32, tag="stats")
                    nc.vector.bn_stats(out=stats[:, 0, :], in_=x_t[:, 0:512])
                    nc.vector.bn_stats(out=stats[:, 1, :],
                                       in_=x_t[:, 512:1024])
                    nc.vector.bn_aggr(out=mv_all[:, pi, :], in_=stats)

                # batched rsqrt: rstd = exp(-0.5*ln(var+eps))
                rstd4 = small_pool.tile([128, 4], f32, tag="rstd")
                nc.scalar.activation(
                    out=rstd4, in_=mv_all[:, :, 1],
                    func=mybir.ActivationFunctionType.Ln, bias=eps_t,
                )
                nc.scalar.activation(
                    out=rstd4, in_=rstd4,
                    func=mybir.ActivationFunctionType.Exp, scale=-0.5,
                )
                nm4 = small_pool.tile([128, 4], f32, tag="nm")
                nc.vector.tensor_tensor(
                    out=nm4, in0=mv_all[:, :, 0], in1=rstd4,
                    op=mybir.AluOpType.mult,
                )
                nc.vector.tensor_scalar_mul(out=nm4, in0=nm4, scalar1=-1.0)

                for pi in range(4):
                    t = s * 4 + pi
                    out_t = out_pool.tile([128, DM], f32, tag="out")
                    if pi % 2 == 0:
                        nc.vector.tensor_scalar(
                            out=out_t, in0=x_ts[pi],
                            scalar1=mv_all[:, pi, 0:1],
                            scalar2=rstd4[:, pi:pi + 1],
                            op0=mybir.AluOpType.subtract,
                            op1=mybir.AluOpType.mult,
                        )
                    else:
                        nc.scalar.activation(
                            out=out_t, in_=x_ts[pi],
                            func=mybir.ActivationFunctionType.Identity,
                            bias=nm4[:, pi:pi + 1],
                            scale=rstd4[:, pi:pi + 1],
                        )
                    if affine:
                        nc.vector.tensor_mul(out_t, out_t, gam_bc)
                        nc.vector.tensor_add(out_t, out_t, bet_bc)
                    nc.gpsimd.dma_start(
                        out=out_d[t * 128:(t + 1) * 128, :], in_=out_t
                    )
                del state[s]

            # software pipeline: epilogue runs one slice behind attention
            for s in range(NSLICES + 1):
                if s < NSLICES:
                    oa = attention(s)
                if s >= 1:
                    epilogue(s - 1)
                if s < NSLICES:
                    dance(s, oa)

    nc.finalize()
    return nc


LAST_RESULTS = None


def kernel(q, k, v, residual, fc_w, fc_b, ln_gamma, ln_beta):
    from concourse.bass_utils import run_bass_kernel_spmd

    global LAST_RESULTS
    affine = not (
        np.allclose(ln_gamma, 1.0) and np.allclose(ln_beta, 0.0)
    )
    packed_fc = bool(np.all(np.asarray(fc_b) == 0.0))
    key = ("v5", affine, packed_fc)
    if key not in _CACHE:
        _CACHE[key] = _build(affine, packed_fc)
    nc = _CACHE[key]

    q = np.ascontiguousarray(q, dtype=np.float32)
    k = np.ascontiguousarray(k, dtype=np.float32)
    v = np.ascontiguousarray(v, dtype=np.float32)
    residual = np.ascontiguousarray(residual, dtype=np.float32)
    fc_w = np.ascontiguousarray(fc_w, dtype=np.float32)
    fc_b = np.ascontiguousarray(fc_b, dtype=np.float32)
    ln_gamma = np.ascontiguousarray(ln_gamma, dtype=np.float32)
    ln_beta = np.ascontiguousarray(ln_beta, dtype=np.float32)

    in_maps = [
        {
            "q": q[b], "k": k[b], "v": v[b], "residual": residual[b],
            "fc_w": fc_w, "fc_b": fc_b,
            "ln_gamma": ln_gamma, "ln_beta": ln_beta,
        }
        for b in range(B)
    ]
    res = run_bass_kernel_spmd(nc, in_maps, core_ids=list(range(B)))
    LAST_RESULTS = res
    return np.stack([res.results[b]["out"] for b in range(B)], axis=0)
